# revision 26
# baseline (speedup 1.0000x reference)
"""Trainium2 Bass kernel for nn_BinaryConv2d (B=16, C=64, H=W=256, 3x3, pad 1).

Forward semantics (STE forward values):
  act = sign(x * rd_k + rd_b)                  in {-1, 0, +1}
  bw  = scaling[co] * sign(conv_w)             scaling = mean |conv_w| per out-ch
  y   = conv2d(act, bw, pad=1)
  y   = prelu(y + pr_bias0) + pr_bias1 + x     prelu slope per channel

Strategy: data-parallel over batch, 2 images per core (8 cores).  The two
images' 64 channels are stacked on the 128 SBUF partitions.  Activations are
binarized to fp8 +-1 on the Scalar engine; the 3x3 conv is 9 accumulating
PSUM matmuls per 2-row pair with block-diagonal +-1 fp8 weights (exact
integer arithmetic in fp32 PSUM), kh=0/1 packed into DoubleRow matmuls.
x ships bf16 with pr_bias1 pre-folded (halves load DMA; the kernel is
otherwise ~15% over the per-core DMA budget), y stores bf16 and is widened
to f32 on the host (~5.6e-3 scaled absmax vs the 2e-2 gate).
"""

import sys

if "/opt/trn_rl_repo" not in sys.path:
    sys.path.insert(0, "/opt/trn_rl_repo")

from contextlib import ExitStack

import ml_dtypes
import numpy as np

import concourse.bacc as bacc
import concourse.bass as bass
import concourse.tile as tile
from concourse import mybir
from concourse.bass_utils import run_bass_kernel_spmd

B, C, H, W = 16, 64, 256, 256
NCORES = 8
HS = 32                      # output rows per strip
NSTRIPS = H // HS
P = 128                      # partitions = 2 images x 64 channels

F32 = mybir.dt.float32
BF16 = mybir.dt.bfloat16
FP8 = mybir.dt.float8e4
AF = mybir.ActivationFunctionType
ALU = mybir.AluOpType

# 'bf16': 9 matmuls (K=128) per 2-row tile on bf16 +-1 operands.
# 'fp8dr': activations/weights in fp8e4; the kh=0/1 pairs are packed into
#   DoubleRow matmuls (2 MACs/cell/cycle), cutting PE streaming ~29%.
#   All values are exactly representable (+-1, 0), so precision is identical.
CONV_MODE = "fp8dr"
APITCH = 272                 # act row pitch (bytes %16 for DoubleRow AP steps)

# Param table columns (per-partition f32 scalars)
PK, PB, PS, PB0, PCM, PB1, PSL = 0, 1, 2, 3, 4, 5, 6

# The ACT-engine Lrelu activation computes something other than
# prelu(x, alpha) on TRN2 hardware (measured absmax 0.1 vs reference), so the
# PReLU is done on the Vector engine instead.
USE_LRELU = False

# prelu(u) == max(u, slope*u) when 0 <= slope <= 1 (checked at runtime in
# kernel()); one fused scalar_tensor_tensor op instead of tensor_scalar +
# tensor_tensor.  Set False for the general min/mult path.
PRELU_MAX_TRICK = True

SIGN_CHUNK = 8               # rows of sign-activation per ACT instruction
STRIP_HS = [32] * 7 + [24, 8]   # strip heights (sum == H); tiny tail strip


def _emit(tc, nc, x_d, w_d, p_d, y_d):
    """Engine assignment:
      ACT     sign + psum evac (Identity, 4-row) for all but the last group
              of each strip, with next-strip sign chunks INTERLEAVED between
              evacs in FIFO order (one chunk per 4-row group) so neither
              starves the PE at strip boundaries; also the store-DMA ring
      DVE     last-group evac + prelu stt (8-row, out-of-place) + residual
              tensor_tensor (all-bf16: 2x DVE mode; b1 pre-folded into x)
      GPSIMD  pad memsets only.  GPSIMD elementwise measured 4.5us/8-row
              (2x engine slowness + it contends with DVE for the shared
              SBUF port, inflating DVE stt 2134->3857ns); its SWDGE store
              path costs a 4.5us drain at teardown -- do NOT use either
      PE      9-matmul fp8-DR pattern per 2-row pair into 4-row psum tiles
              (2 banks, bufs=4; bufs=2 with 8-row tiles exposed ACT FIFO
              latency as 38us of PE stalls), HAM pre-warmed by 40 dummy
              matmuls so the stream runs at 2.4 GHz from the first row
    Loads keep the sync ring exclusive (a store queued in front of a strip
    load delays next-strip sign; the PE laps DMA otherwise).  Warm-state
    PE floor is ~164us streaming; measured stream is ~169us with zero
    gaps >0.8us.  Run-to-run: the chip drops to ~2.0 GHz under sustained
    power (P0), inflating exec ~19%; idle cool-down restores it."""
    x3 = x_d.rearrange("p (h w) -> p h w", w=W)
    y3 = y_d.rearrange("p (h w) -> p h w", w=W)
    fp8dr = CONV_MODE == "fp8dr"
    adt = FP8 if fp8dr else BF16
    apitch = APITCH if fp8dr else W + 2

    H0S = [sum(STRIP_HS[:i]) for i in range(len(STRIP_HS))]
    NST = len(STRIP_HS)
    HSMAX = max(STRIP_HS)

    with ExitStack() as ctx:
        consts = ctx.enter_context(tc.tile_pool(name="consts", bufs=1))
        xpool = ctx.enter_context(tc.tile_pool(name="xpool", bufs=3))
        ypool = ctx.enter_context(tc.tile_pool(name="ypool", bufs=3))
        vpool = ctx.enter_context(tc.tile_pool(name="vpool", bufs=2))
        pspool = ctx.enter_context(tc.tile_pool(name="pspool", bufs=4,
                                                space="PSUM"))

        # params first on the load ring (sign needs them); weights on the
        # scalar ring, which is idle at kernel start
        pt = consts.tile([P, 8], F32)
        nc.sync.dma_start(out=pt, in_=p_d)
        if fp8dr:
            # [kw, delta(kh 0/1), m] DoubleRow weights + [kw, m] kh=2 weights
            wdr = consts.tile([P, 3, 2, 128], FP8)
            nc.scalar.dma_start(out=wdr, in_=w_d[:, :768].rearrange(
                "p (k d m) -> p k d m", k=3, d=2))
            wn = consts.tile([P, 3, 128], FP8)
            nc.scalar.dma_start(out=wn, in_=w_d[:, 768:].rearrange(
                "p (k m) -> p k m", k=3))
        else:
            wt = consts.tile([P, 9, 128], BF16)
            nc.scalar.dma_start(out=wt,
                                in_=w_d.rearrange("p (j m) -> p j m", j=9))

        # HAM pre-warm: ~30 back-to-back dummy matmuls during the DMA
        # preamble trip the PE activity monitor's 4096-cycle window, so the
        # real matmuls start at 2.4 GHz instead of ramping from 1.2 GHz
        # ~3.5us in.  Scratch weights are memset on DVE (idle at start);
        # the psum tile is one pool rotation slot, reclaimed via WAR.
        dummy_w = consts.tile([P, 128], FP8, name="dummy_w")
        nc.vector.memset(dummy_w, 0.0)
        # preload the ACT Sign table during the preamble (the implicit
        # ACT_TABLE_LOAD is 1.3us and otherwise lands in the first sign
        # chunk's critical path)
        zs = consts.tile([P, 2], F32, name="zscal")
        nc.vector.memset(zs, 0.0)
        warm_sign = consts.tile([P, 16], FP8, name="warm_sign")
        nc.scalar.activation(warm_sign, dummy_w[:, 0:16], AF.Sign,
                             bias=zs[:, 0:1], scale=zs[:, 1:2])
        # aim at the first real psum tile: its own start=True matmuls
        # overwrite whatever the dummies left there
        ps_first = pspool.tile([P, 4, W], F32, name="ps")
        for _ in range(40):
            nc.tensor.matmul(ps_first[:, 0, 0:128], lhsT=dummy_w,
                             rhs=dummy_w, start=True, stop=True)
        ps_pre = [ps_first]

        # Two persistent act buffers (manual double buffer).  The zero pad
        # columns (0 and W+1) are memset ONCE here; sign never writes them
        # and matmuls only read them, so they stay zero across all strips.
        acts = []
        for i in range(2):
            a = consts.tile([P, HSMAX + 2, apitch], adt, name=f"act{i}")
            nc.gpsimd.memset(a[:, :, 0:1], 0.0)
            nc.gpsimd.memset(a[:, :, W + 1:W + 2], 0.0)
            acts.append(a)
        # strip 0 top pad row (overwritten later by strip-2 sign: harmless,
        # strip 0 has consumed it by then)
        nc.gpsimd.memset(acts[0][:, 0:1, :], 0.0)

        def strip_rows(s):
            h0 = H0S[s]
            row_lo = max(h0 - 1, 0)
            row_hi = min(h0 + STRIP_HS[s] + 1, H)
            return h0, row_lo, row_hi, row_lo - (h0 - 1)

        def load_strip(s):
            """DMA the x strip (rows h0-1 .. h0+hs; tile row a <-> global
            h0-1+a)."""
            h0, row_lo, row_hi, r0 = strip_rows(s)
            xs = xpool.tile([P, HSMAX + 2, W], BF16, name="xs")
            if s == 0:
                # tiny first transfer so the first sign chunk fires early
                bounds = [0, 4, 9, 15, 24, row_hi]
            else:
                nr = row_hi - row_lo
                bounds = [row_lo + (nr * i) // 3 for i in range(4)]
            for a, b in zip(bounds, bounds[1:]):
                if b > a:
                    nc.sync.dma_start(out=xs[:, a - (h0 - 1):b - (h0 - 1), :],
                                      in_=x3[:, a:b, :])
            return xs, acts[s % 2]

        def sign_strip(s, xs, act, chunks, skip=0):
            """Binarize x into the zero-padded act tile, in row chunks (the
            first small so dependent matmuls unblock quickly)."""
            _, row_lo, row_hi, r0 = strip_rows(s)
            c0 = r0 + skip
            for sz in chunks:
                c1 = min(c0 + sz, r0 + (row_hi - row_lo))
                if c1 <= c0:
                    break
                nc.scalar.activation(
                    act[:, c0:c1, 1:W + 1], xs[:, c0:c1, :], AF.Sign,
                    bias=pt[:, PB:PB + 1], scale=pt[:, PK:PK + 1],
                )
                c0 = c1
            if s == NST - 1 and skip == 0:
                # bottom pad row; emitted with the last strip's sign so it
                # lands after the earlier strip that shares this buffer
                nc.gpsimd.memset(act[:, STRIP_HS[s] + 1:STRIP_HS[s] + 2, :],
                                 0.0)

        cur = load_strip(0)
        sign_strip(0, *cur, (4, 5, 6, 9, 9))
        nxt = None
        for s in range(NST):
            h0 = H0S[s]
            HS_S = STRIP_HS[s]
            NG = HS_S // 4           # 4-row psum groups
            xs, act = cur
            ys = None
            v8 = None
            if s + 1 < NST:
                # next-strip sign chunk sizes: one chunk per 4-row group,
                # emitted at groups 1..n so they interleave with this
                # strip's evacs on the ACT FIFO
                nr1 = strip_rows(s + 1)[2] - strip_rows(s + 1)[1]
                nck = []
                left = nr1
                for sz in (5,) + (SIGN_CHUNK,) * 8:
                    if left <= 0:
                        break
                    nck.append(min(sz, left))
                    left -= nck[-1]
                assert len(nck) <= NG - 1
            for g in range(NG):
                if s + 1 < NST and g == 0:
                    nxt = load_strip(s + 1)   # loads overlap this strip
                if s + 1 < NST and 1 <= g <= len(nck):
                    sign_strip(s + 1, *nxt, nck[g - 1:g],
                               skip=sum(nck[:g - 1]))
                ps = ps_pre.pop() if ps_pre else pspool.tile(
                    [P, 4, W], F32, name="ps")
                for half in range(2):
                    r = 4 * g + 2 * half     # first output row of pair
                    po = ps[:, 2 * half:2 * half + 2, :]
                    if fp8dr:
                        for kw in range(3):
                            for i in range(2):
                                # kh in {0,1} via DoubleRow: contraction over
                                # (partition, delta), act row (r+i)+delta
                                nc.tensor.matmul(
                                    po[:, i, :],
                                    lhsT=wdr[:, kw, :, :],
                                    rhs=act[:, r + i:r + i + 2, kw:kw + W],
                                    start=(kw == 0 and i == 0),
                                    stop=False,
                                    perf_mode=mybir.MatmulPerfMode.DoubleRow,
                                )
                        for kw in range(3):
                            # kh=2 plain matmul over both output rows
                            nc.tensor.matmul(
                                po,
                                lhsT=wn[:, kw, :],
                                rhs=act[:, r + 2:r + 4, kw:kw + W],
                                start=False,
                                stop=(kw == 2),
                            )
                    else:
                        for j in range(9):
                            kh, kw = divmod(j, 3)
                            nc.tensor.matmul(
                                po,
                                lhsT=wt[:, j, :],
                                rhs=act[:, r + j // 3:r + j // 3 + 2,
                                        kw:kw + W],
                                start=(j == 0),
                                stop=(j == 8),
                            )
                # --- post-ops ---
                if ys is None or (g * 4) % 16 == 0:
                    yrows = min(16, HS_S - 4 * g)
                    ys = ypool.tile([P, 16, W], BF16, name="ys")
                    y0 = 4 * g       # strip row of ys[0]
                if s == NST - 1:
                    # final strip: 2-row evac/prelu/residual/store chains so
                    # only the very last 2-row chain trails the last matmul
                    v8 = vpool.tile([P, 8, W], F32, name="v")
                    for hf in range(2):
                        r2 = 4 * g + 2 * hf
                        v2 = v8[:, 2 * hf:2 * hf + 2, :]
                        nc.scalar.activation(
                            v2, ps[:, 2 * hf:2 * hf + 2, :], AF.Identity,
                            bias=pt[:, PB0:PB0 + 1], scale=pt[:, PS:PS + 1])
                        u2 = ys[:, r2 - y0:r2 - y0 + 2, :]
                        x2 = xs[:, r2 + 1:r2 + 3, :]
                        if PRELU_MAX_TRICK:
                            nc.vector.scalar_tensor_tensor(
                                u2, v2, pt[:, PSL:PSL + 1], v2,
                                ALU.mult, ALU.max)
                        else:
                            m = vpool.tile([P, 8, W], F32,
                                           name="m8")[:, :2, :]
                            nc.vector.tensor_scalar(
                                m, v2, 0.0, pt[:, PCM:PCM + 1],
                                ALU.min, ALU.mult)
                            nc.vector.tensor_tensor(u2, v2, m, ALU.add)
                        nc.vector.tensor_tensor(u2, x2, u2, ALU.add)
                        nc.scalar.dma_start(
                            out=y3[:, h0 + r2:h0 + r2 + 2, :], in_=u2)
                    continue
                # evac at 4-row granularity into an 8-row v tile: ACT for
                # all but the strip's last group (interleaves with sign on
                # the ACT FIFO), DVE for the last (dodges the strip-tail
                # bunching behind the final sign chunks).  Last two strips
                # have no sign left, so everything goes to ACT for maximum
                # tail parallelism.
                if g % 2 == 0:
                    v8 = vpool.tile([P, 8, W], F32, name="v")
                half4 = (g % 2) * 4
                if g == NG - 1 and s < NST - 2:
                    nc.vector.tensor_scalar(
                        v8[:, half4:half4 + 4, :], ps,
                        pt[:, PS:PS + 1], pt[:, PB0:PB0 + 1],
                        ALU.mult, ALU.add)
                else:
                    nc.scalar.activation(
                        v8[:, half4:half4 + 4, :], ps, AF.Identity,
                        bias=pt[:, PB0:PB0 + 1], scale=pt[:, PS:PS + 1])
                if g % 2 == 1 or g == NG - 1:
                    # flush the filled (possibly half) v tile: prelu +
                    # residual at up-to-8-row granularity
                    nr8 = half4 + 4
                    r8 = 4 * (g - (g % 2))
                    vv = v8[:, :nr8, :]
                    u8 = ys[:, r8 - y0:r8 - y0 + nr8, :]
                    x8 = xs[:, r8 + 1:r8 + 1 + nr8, :]
                    if PRELU_MAX_TRICK:
                        # out-of-place into ys (in-place stt measured slow)
                        nc.vector.scalar_tensor_tensor(
                            u8, vv, pt[:, PSL:PSL + 1], vv, ALU.mult, ALU.max)
                    else:
                        m = vpool.tile([P, 8, W], F32, name="m8")[:, :nr8, :]
                        nc.vector.tensor_scalar(
                            m, vv, 0.0, pt[:, PCM:PCM + 1], ALU.min, ALU.mult)
                        nc.vector.tensor_tensor(u8, vv, m, ALU.add)
                    # residual: b1 pre-folded into x, so a plain all-bf16
                    # add (2x DVE mode)
                    nc.vector.tensor_tensor(u8, x8, u8, ALU.add)
                    if (r8 + nr8 - y0) == 16 or r8 + nr8 == HS_S:
                        # store the filled ys tile on the scalar HWDGE
                        # ring (gpsimd SWDGE costs a 4.5us drain at
                        # teardown; loads keep sync exclusive)
                        nc.scalar.dma_start(
                            out=y3[:, h0 + y0:h0 + y0 + yrows, :],
                            in_=ys[:, :yrows, :])
            cur = nxt


def build_nc():
    nc = bacc.Bacc("TRN2", target_bir_lowering=False, debug=False,
                   num_devices=NCORES)
    wdt = FP8 if CONV_MODE == "fp8dr" else BF16
    x_d = nc.dram_tensor("xin", [P, H * W], BF16, kind="ExternalInput").ap()
    w_d = nc.dram_tensor("wp", [P, 9 * 128], wdt, kind="ExternalInput").ap()
    p_d = nc.dram_tensor("pp", [P, 8], F32, kind="ExternalInput").ap()
    y_d = nc.dram_tensor("yout", [P, H * W], BF16, kind="ExternalOutput").ap()
    with tile.TileContext(nc) as tc:
        _emit(tc, nc, x_d, w_d, p_d, y_d)
    nc.compile()
    return nc


_NC_CACHE = {}


def _get_nc():
    key = (USE_LRELU, PRELU_MAX_TRICK, CONV_MODE)
    if key not in _NC_CACHE:
        _NC_CACHE[key] = build_nc()
    return _NC_CACHE[key]


def make_inputs(x, rd_k, rd_b, beta, conv_w, pr_bias0, prelu_w, pr_bias1):
    """Host-side prep: per-channel param table, packed sign weights, shards."""
    k = np.asarray(rd_k, np.float32).reshape(C)
    b = np.asarray(rd_b, np.float32).reshape(C)
    s = np.mean(np.abs(np.asarray(conv_w, np.float32)), axis=(1, 2, 3))
    b0 = np.asarray(pr_bias0, np.float32).reshape(C)
    slope = np.asarray(prelu_w, np.float32).reshape(C)
    b1 = np.asarray(pr_bias1, np.float32).reshape(C)
    cm = slope - 1.0
    # b1 is folded into x on the host (x' = x + b1): the residual becomes a
    # plain add (u + x'), and the sign bias compensates: k*x' + (b - k*b1)
    # == k*x + b.
    badj = b - k * b1
    cols = np.stack([k, badj, s, b0, cm, b1, slope, np.zeros(C, np.float32)],
                    axis=1)
    pp = np.concatenate([cols, cols], axis=0).astype(np.float32)  # [128, 8]

    sw = np.sign(np.asarray(conv_w, np.float32)).astype(np.float32)  # [co,ci,kh,kw]

    def blockdiag(kh, kw):
        S = sw[:, :, kh, kw].T  # [ci, co]
        out = np.zeros((P, P), np.float32)
        out[0:C, 0:C] = S
        out[C:P, C:P] = S
        return out

    if CONV_MODE == "fp8dr":
        wp = np.zeros((P, 9, 128), np.float32)
        for kw in range(3):            # [kw, delta, m] DoubleRow pairs
            for d in range(2):
                wp[:, kw * 2 + d, :] = blockdiag(d, kw)
        for kw in range(3):            # [kw, m] kh=2
            wp[:, 6 + kw, :] = blockdiag(2, kw)
        wdt = mybir.dt.np(FP8)
    else:
        wp = np.zeros((P, 9, 128), np.float32)
        for j in range(9):
            kh, kw = divmod(j, 3)
            wp[:, j, :] = blockdiag(kh, kw)
        wdt = ml_dtypes.bfloat16
    wp = np.ascontiguousarray(wp.reshape(P, 9 * 128)).astype(wdt)

    x = np.asarray(x, np.float32)
    if np.any(b1 != 0.0):
        x = x + b1[None, :, None, None]
    # x ships as bf16: halves load DMA (the kernel is otherwise ~15% over
    # the per-core DMA budget in steady state).  Sign is preserved (bf16
    # rounding never crosses zero); the residual add absorbs a <=0.4%
    # relative error on x, ~5e-4 of the output scale.
    x = x.astype(ml_dtypes.bfloat16)
    in_maps = []
    for c in range(NCORES):
        xc = np.ascontiguousarray(x[2 * c:2 * c + 2]).reshape(P, H * W)
        in_maps.append({"xin": xc, "wp": wp, "pp": pp})
    return in_maps


def kernel(x, rd_k, rd_b, beta, conv_w, pr_bias0, prelu_w, pr_bias1):
    global PRELU_MAX_TRICK
    slope = np.asarray(prelu_w, np.float32).reshape(C)
    if not np.all((slope >= 0.0) & (slope <= 1.0)):
        PRELU_MAX_TRICK = False   # max-identity only valid for slope in [0,1]
    in_maps = make_inputs(x, rd_k, rd_b, beta, conv_w, pr_bias0, prelu_w,
                          pr_bias1)
    nc = _get_nc()
    res = run_bass_kernel_spmd(nc, in_maps, core_ids=list(range(NCORES)))
    y = np.empty((B, C, H, W), np.float32)
    for c in range(NCORES):
        y[2 * c:2 * c + 2] = np.asarray(
            res.results[c]["yout"], dtype=np.float32).reshape(2, C, H, W)
    return y

